# revision 19
# baseline (speedup 1.0000x reference)
"""Single-head causal attention with RoPE + padding mask, data-parallel
over batch across 8 TRN2 NeuronCores (one batch element per core).

Per core (T=4096, C=128, HS=64):
  q = rope(x @ Wq); k = rope(x @ Wk); v = x @ Wv
  S^T[j,i] = k[j]·q[i]           (scores, transposed layout: partition=j)
  P^T = exp(S^T/sqrt(C)) * tri(i>=j)   (no max-subtraction: scores are
        O(0.1) for this problem so exp is numerically safe)
  outT[d,i] = sum_j (mask[j]*v[j,d]) P^T[j,i]; rowsum via a mask column
        appended to v (padding mask applied on the v/rowsum side)
  out[i,d] = outT[d,i] / rowsum[i]

V2 structure — the ScalarE exp stream is the bottleneck (1 elem/lane/cyc
@1.2GHz, ~261ns fixed overhead per ACTIVATE), so everything is organized
to minimize exp'd columns and ACTIVATE count and keep ScalarE saturated:
  - Diagonal-band j-tiles are column-sliced to their causal extent
    (512/384/256/128 for tt=0..3), cutting exp'd cols to the causal
    floor 67584 (vs 73728) and shrinking S/PV matmuls the same way.
  - Scores pack as variable-width "atoms" into two alternating PSUM
    score buffers [128,1536]/[128,1024] -> ~56 big ACTIVATEs instead of
    72x1024.  Atom offsets never cross a PSUM bank boundary.
  - PSUM budget (8 banks): sgA 3 + sgB 2 + outT 1 + proj 2.
  - Head: input DMAs split across sync+scalar queues, rope(0) chain is
    the only thing between DMA land and the first exp.
  - bf16 TensorE compute, fp32 PSUM accumulate; cos/sin shipped bf16.
  - S^T matmuls row-packed in concurrent pairs via tile_position
    (0,0)/(64,0) with q/k duplicated into partitions 64-127.
"""

import numpy as np

T, C, HS = 4096, 128, 64
N_CORES = 8
NT = T // 128      # 32 j-tiles of 128
NCH = T // 512     # 8 i-chunks of 512
CAP_A = 1536       # score-group buffer A capacity (3 PSUM banks)
CAP_B = 1024       # score-group buffer B capacity (2 PSUM banks)
SCALE = float(1.0 / np.sqrt(np.float32(C)))

_CACHE = {}


def _install_tile_drain_patch(tile_mod):
    """This container's walrus rejects instructions with >2 sem waits; split
    Tile's final global drain into one drain per ticked processor."""
    import bass_rust
    from concourse.vector_clock import ScopedClock

    def _patched(self, tick_clock, wait_clock):
        gc = tick_clock.global_clock
        for i in range(len(gc)):
            if gc[i] <= 0:
                continue
            v = bass_rust.VectorClock()
            v.require_at_least(i, gc[i])
            d = self.nc.sync.drain()
            wait_clock.add_sem_waits(d.ins, ScopedClock({None: v}))
        self.nc.all_engine_barrier()
        assert self.sems is not None
        popped = self.nc._tile_sem_poison_stack.pop()
        assert popped is self._sem_poison
        self.nc.clear_and_free_semaphores(list(self.sems.allocated().values()))
        self.nc.all_engine_barrier()

    tile_mod.TileContext._drain_and_barrier = _patched


def _split_excess_waits(nc, mybir, limit=1):
    """Hoist excess sem waits onto standalone EventSemaphore instructions."""
    ctr = 0
    for f in nc.m.functions:
        for b in f.blocks:
            il = b.instructions
            out = []
            changed = False
            for ins in il:
                si = ins.sync_info
                waits = list(si.on_wait) if si and si.on_wait else []
                if len(waits) > limit:
                    changed = True
                    excess = waits[: len(waits) - limit]
                    keep = waits[len(waits) - limit :]
                    for i in range(0, len(excess), limit):
                        chunk = excess[i : i + limit]
                        ev = mybir.InstEventSemaphore(
                            name=f"I-waitsplit-{ctr}",
                            engine=ins.engine,
                            ins=[],
                            outs=[],
                            sync_info=mybir.SyncInfo(on_wait=chunk, on_update=[]),
                        )
                        ctr += 1
                        nc.register_instruction(ev)
                        out.append(ev)
                    si.on_wait = keep
                out.append(ins)
            if changed:
                b.instructions = out


def _chunk_atoms(ic):
    """Atom list for i-chunk ic: (jt, pv_colbase, width).
    Full-width tiles first (nondiag j-tiles plus the tt0 diagonal tile,
    all width 512), then the sliced diagonal tiles in order tt1, tt3,
    tt2 so running offsets never cross a 512 (bank) boundary."""
    atoms = []
    for jt in range(4 * ic + 1):          # jt = 4*ic is the tt0 diag tile
        atoms.append((jt, 0, 512))
    if SLICE:
        atoms.append((4 * ic + 1, 128, 384))  # tt1
        atoms.append((4 * ic + 3, 384, 128))  # tt3
        atoms.append((4 * ic + 2, 256, 256))  # tt2
    else:
        atoms.append((4 * ic + 1, 0, 512))
        atoms.append((4 * ic + 2, 0, 512))
        atoms.append((4 * ic + 3, 0, 512))
    return atoms


def _build_groups(ic, parity):
    """Greedy pack chunk atoms into alternating A/B score buffers.
    Returns (groups, parity) where each group is (use_a, [(jt, colbase,
    width, off)])."""
    atoms = _chunk_atoms(ic)
    if ic == 0:
        # two groups so the first exp waits on one matmul only
        g0 = [(0, 0, 512, 0)]
        g1 = [(1, 128, 384, 0), (3, 384, 128, 384), (2, 256, 256, 512)]
        return [(parity == 0, g0), (parity == 1, g1)], parity
    groups = []
    cur, off = [], 0
    cap = CAP_A if parity == 0 else CAP_B
    for jt, colbase, w in atoms:
        if off + w > cap:
            groups.append((parity == 0, cur))
            parity ^= 1
            cur, off = [], 0
            cap = CAP_A if parity == 0 else CAP_B
        cur.append((jt, colbase, w, off))
        off += w
    if cur:
        groups.append((parity == 0, cur))
        parity ^= 1
    return groups, parity


def _build_nc():
    import concourse.bass as bass
    import concourse.mybir as mybir
    from concourse import tile, masks

    _install_tile_drain_patch(tile)

    DT = mybir.dt
    F32, BF16 = DT.float32, DT.bfloat16
    AF = mybir.ActivationFunctionType
    ALU = mybir.AluOpType

    nc = bass.Bass()
    xT_e = nc.declare_dram_parameter("xT", [C, T], BF16, isOutput=False)
    # w packed flat: [C, 320] = [Wq | Wq_swap | Wk | Wk_swap | Wv]
    w_e = nc.declare_dram_parameter("w", [C, 5 * HS], BF16, isOutput=False)
    # cs2: rows 0-63 = cos2, rows 64-127 = sin2s (signed), [128, T]
    cs2_e = nc.declare_dram_parameter("cs2", [128, T], BF16, isOutput=False)
    mask01_e = nc.declare_dram_parameter("mask01", [128, NT], F32, isOutput=False)
    # out in [t%128, t//128, d] layout; host reassembles to [T, HS]
    out_e = nc.declare_dram_parameter("out", [128, NT, HS], F32, isOutput=True)

    with tile.TileContext(nc) as tc:
        with (
            tc.tile_pool(name="const", bufs=1) as cpool,
            tc.tile_pool(name="work", bufs=3) as wpool,
            tc.tile_pool(name="ps", bufs=2, space="PSUM") as ps,
        ):
            xT = cpool.tile([C, T], BF16)
            w_sb = cpool.tile([C, 5 * HS], BF16)
            mask01 = cpool.tile([128, NT], F32)
            cs2 = cpool.tile([128, T], BF16)

            # preload the exp table: a dummy activation issued before
            # anything else pulls ACT_TABLE_LOAD into the preamble
            scr = cpool.tile([1, 2], F32)
            nc.vector.memset(scr[:, :], 0.0)
            scr2 = cpool.tile([1, 2], F32)
            nc.scalar.activation(scr2[:, :], scr[:, :], AF.Exp, bias=0.0, scale=1.0)

            # ---- input DMAs, all on sync: chunk-0/1 deps first ----
            def in_dma(ch, eng):
                sl = slice(ch * 512, (ch + 1) * 512)
                eng.dma_start(out=xT[:, sl], in_=xT_e[:, sl])
                eng.dma_start(out=cs2[:, sl], in_=cs2_e[:, sl])

            nc.sync.dma_start(out=w_sb[:, :], in_=w_e[:, :])
            nc.sync.dma_start(out=xT[:, 0:512], in_=xT_e[:, 0:512])
            nc.gpsimd.dma_start(out=cs2[:, 0:512], in_=cs2_e[:, 0:512])
            nc.gpsimd.dma_start(out=cs2[:, 512:1024], in_=cs2_e[:, 512:1024])
            nc.sync.dma_start(out=xT[:, 512:1024], in_=xT_e[:, 512:1024])
            nc.sync.dma_start(out=mask01[:, :], in_=mask01_e[:, :])
            for ch in range(2, NCH):
                in_dma(ch, nc.sync)

            identity = cpool.tile([128, 128], F32)
            masks.make_identity(nc, identity[:, :])

            # q2/k2: rows 0..63 = rope(q/k)^T, rows 64..127 duplicate for
            # row-packed (tile_position) S matmuls
            q2 = cpool.tile([128, T], BF16)
            k2 = cpool.tile([128, T], BF16)

            # v tiles + mask column (mask-weighted rowsum): [t, j_tile, d(65)]
            vplus = cpool.tile([128, NT, HS + 1], BF16)
            nc.vector.tensor_copy(vplus[:, :, HS], mask01[:, :])

            out_stage = cpool.tile([128, NT, HS], F32)

            def rope_block(ch):
                # merged: one [128,512] matmul per head side (q rows 0-63,
                # q_swap rows 64-127), one mul with stacked cos|sin, then a
                # partition-realign DMA + add + duplicate-rows DMA
                sl = slice(ch * 512, (ch + 1) * 512)
                if ch < 2:
                    # head-latency-critical: unmerged keeps the chain free of
                    # SBUF->SBUF DMA hops (only the row-dup DMA remains; the
                    # first score groups are solos that don't need it)
                    for nm, wlo, dst in (("q", 0, q2), ("k", 128, k2)):
                        pa = ps.tile([HS, 512], F32, tag="proj", bufs=2,
                                     name=f"{nm}a_ps{ch}")
                        nc.tensor.matmul(pa[:, :], w_sb[:, wlo : wlo + HS],
                                         xT[:, sl], start=True, stop=True)
                        pb = ps.tile([HS, 512], F32, tag="proj", bufs=2,
                                     name=f"{nm}b_ps{ch}")
                        nc.tensor.matmul(pb[:, :], w_sb[:, wlo + HS : wlo + 128],
                                         xT[:, sl], start=True, stop=True)
                        ma = wpool.tile([HS, 512], BF16, tag="rope", bufs=4,
                                        name=f"ma_{nm}{ch}")
                        nc.vector.tensor_mul(ma[:, :], pa[:, :], cs2[0:HS, sl])
                        mb = wpool.tile([HS, 512], BF16, tag="rope", bufs=4,
                                        name=f"mb_{nm}{ch}")
                        nc.vector.tensor_mul(mb[:, :], pb[:, :], cs2[64:128, sl])
                        nc.vector.tensor_add(dst[0:HS, sl], ma[:, :], mb[:, :])
                        nc.gpsimd.dma_start(out=dst[64:128, sl], in_=dst[0:64, sl])
                    return
                for nm, wlo, dst in (("q", 0, q2), ("k", 128, k2)):
                    p_ps = ps.tile([128, 512], F32, tag="proj", bufs=2,
                                   name=f"{nm}_ps{ch}")
                    nc.tensor.matmul(p_ps[:, :], w_sb[:, wlo : wlo + 128],
                                     xT[:, sl], start=True, stop=True)
                    m = wpool.tile([128, 512], BF16, tag="rope", bufs=4,
                                   name=f"m_{nm}{ch}")
                    nc.vector.tensor_mul(m[:, :], p_ps[:, :], cs2[:, sl])
                    mlo = wpool.tile([64, 512], BF16, tag="ropelo", bufs=4,
                                     name=f"mlo_{nm}{ch}")
                    nc.gpsimd.dma_start(out=mlo[:, :], in_=m[64:128, :])
                    nc.vector.tensor_add(dst[0:HS, sl], m[0:64, :], mlo[:, :])
                    nc.gpsimd.dma_start(out=dst[64:128, sl], in_=dst[0:64, sl])

            def v_block(ch):
                for tt in range(4):
                    jt = ch * 4 + tt
                    v_ps = ps.tile([128, HS], F32, tag="proj", bufs=2, name=f"v_ps{jt}")
                    nc.tensor.matmul(
                        v_ps[:, :],
                        xT[:, jt * 128 : (jt + 1) * 128],
                        w_sb[:, 256:320],
                        start=True,
                        stop=True,
                    )
                    nc.vector.tensor_scalar_mul(
                        vplus[:, jt, 0:HS], v_ps[:, :], mask01[:, jt : jt + 1]
                    )

            # minimal head: rope(0) feeds the first scores; rope(1)/v right
            # behind on the queues
            rope_block(0)
            rope_block(1)
            v_block(0)
            v_block(1)

            def epilogue_copy(ic, outT_ps):
                outT_sb = wpool.tile([HS + 1, 512], F32, tag="outTsb", bufs=2,
                                     name=f"oT{ic}")
                nc.vector.tensor_copy(outT_sb[:, 0:256], outT_ps[:, 0:256])
                nc.vector.tensor_copy(outT_sb[:, 256:512], outT_ps[:, 256:512])
                return outT_sb

            def epilogue_tr(ic, outT_sb):
                for tt in range(4):
                    jt = ic * 4 + tt
                    tr_ps = ps.tile([128, HS + 1], F32, tag="proj", bufs=2,
                                    name=f"tr{jt}")
                    nc.tensor.matmul(
                        tr_ps[:, :],
                        outT_sb[:, tt * 128 : (tt + 1) * 128],
                        identity[0 : HS + 1, 0 : HS + 1],
                        is_transpose=True,
                        start=True,
                        stop=True,
                    )
                    recip = wpool.tile([128, 1], F32, tag="recip", bufs=8)
                    nc.vector.reciprocal(recip[:, :], tr_ps[:, HS : HS + 1])
                    nc.vector.tensor_scalar_mul(
                        out_stage[:, jt, :], tr_ps[:, 0:HS], recip[:, :]
                    )
                nc.sync.dma_start(
                    out=out_e[:, ic * 4 : ic * 4 + 4, :],
                    in_=out_stage[:, ic * 4 : ic * 4 + 4, :],
                )

            # flat group list + per-group weave assignments
            parity = 0
            all_groups = []  # (ic, use_a, atoms, chunk_last)
            for ic in range(NCH):
                groups, parity = _build_groups(ic, parity)
                for gi, (use_a, atoms) in enumerate(groups):
                    all_groups.append((ic, use_a, atoms, gi == len(groups) - 1))
            G = len(all_groups)
            # distribute weave work (proj two chunks ahead, transpose
            # epilogue one chunk behind) across each chunk's group slots
            inserts = {}
            gidx_of_chunk = {}
            for idx, (ic, _, _, _) in enumerate(all_groups):
                gidx_of_chunk.setdefault(ic, []).append(idx)
            for ic in range(NCH):
                slots = gidx_of_chunk[ic]
                work = []
                if ic == 0:
                    work.append(("proj", 2))
                    work.append(("proj", 3))
                elif ic + 3 < NCH:
                    work.append(("proj", ic + 3))
                if ic >= 1:
                    work.append(("tr", ic - 1))
                for i, wk in enumerate(work):
                    slot = slots[min(i * max(1, len(slots) // 2), len(slots) - 1)]
                    inserts.setdefault(slot, []).append(wk)

            outT_of = {}
            sb_of = {}

            def issue_scores(idx):
                ic, use_a, atoms, _ = all_groups[idx]
                isl_base = ic * 512
                cap = CAP_A if use_a else CAP_B
                tag = "sga" if use_a else "sgb"
                sg = ps.tile([128, cap], F32, tag=tag, bufs=1,
                             name=f"sg{ic}_{atoms[0][0]}")

                def s_mm(atom, ro):
                    jt, colbase, w, off = atom
                    nc.tensor.matmul(
                        sg[:, off : off + w],
                        k2[ro : ro + HS, jt * 128 : (jt + 1) * 128],
                        q2[ro : ro + HS, isl_base + colbase : isl_base + colbase + w],
                        start=True,
                        stop=True,
                        tile_position=(ro, 0),
                    )

                # row-packed pairs: equal width + different PSUM banks only
                pend = None
                for atom in atoms:
                    if ic < 2:
                        s_mm(atom, 0)
                        continue
                    if pend is not None and pend[3] // 512 != atom[3] // 512 and pend[2] == atom[2]:
                        s_mm(pend, 0)
                        s_mm(atom, 64)
                        pend = None
                    elif pend is not None:
                        s_mm(pend, 0)
                        pend = atom
                    else:
                        pend = atom
                if pend is not None:
                    s_mm(pend, 0)
                return sg

            def issue_exp(idx, sg):
                ic, use_a, atoms, _ = all_groups[idx]
                used = atoms[-1][3] + atoms[-1][2]
                pt = wpool.tile([128, CAP_A], BF16, tag="pt", bufs=4,
                                name=f"pt{ic}_{atoms[0][0]}")
                nc.scalar.activation(pt[:, 0:used], sg[:, 0:used], AF.Exp,
                                     bias=0.0, scale=SCALE)
                return pt

            def issue_tail(idx, pt):
                ic, use_a, atoms, chunk_last = all_groups[idx]
                outT_ps = outT_of[ic]
                for jt, colbase, w, off in atoms:
                    if jt >= 4 * ic:
                        nc.gpsimd.affine_select(
                            out=pt[:, off : off + w],
                            in_=pt[:, off : off + w],
                            compare_op=ALU.is_ge,
                            fill=0.0,
                            base=colbase - 128 * (jt - 4 * ic),
                            pattern=[[1, w]],
                            channel_multiplier=-1,
                        )
                    nc.tensor.matmul(
                        outT_ps[:, colbase : colbase + w],
                        vplus[:, jt, :],
                        pt[:, off : off + w],
                        start=(jt == 0 and colbase == 0),
                        stop=(jt == 4 * ic + 2),
                    )
                if chunk_last:
                    sb_of[ic] = epilogue_copy(ic, outT_ps)
                for wk in inserts.get(idx, []):
                    if wk[0] == "proj":
                        rope_block(wk[1])
                        v_block(wk[1])
                    else:
                        epilogue_tr(wk[1], sb_of[wk[1]])

            # software pipeline: scores one group ahead of exp, tails two
            # behind — everything on the PE queue is dependency-free by the
            # time the engine reaches it
            outT_of[0] = ps.tile([HS + 1, 512], F32, tag="outT", bufs=1, name="outT0")
            sg_cur = issue_scores(0)
            pts = {}
            for idx in range(G):
                pts[idx] = issue_exp(idx, sg_cur)
                if idx + 1 < G:
                    nic = all_groups[idx + 1][0]
                    if nic not in outT_of:
                        outT_of[nic] = ps.tile([HS + 1, 512], F32, tag="outT",
                                               bufs=1, name=f"outT{nic}")
                    sg_cur = issue_scores(idx + 1)
                if idx - 1 >= 0:
                    issue_tail(idx - 1, pts.pop(idx - 1))
            issue_tail(G - 1, pts.pop(G - 1))
            epilogue_tr(NCH - 1, sb_of[NCH - 1])

    _split_excess_waits(nc, mybir, limit=1)
    return nc


def _get_nc():
    if "nc" not in _CACHE:
        _CACHE["nc"] = _build_nc()
    return _CACHE["nc"]


def kernel(x_text_emb, Wq, Wk, Wv, freqs_cos, freqs_sin, x_latex_mask):
    import ml_dtypes
    from concourse.bass_utils import run_bass_kernel_spmd

    bf16 = ml_dtypes.bfloat16
    nc = _get_nc()

    swap = np.arange(HS) ^ 1
    cos2 = np.repeat(np.asarray(freqs_cos, np.float32).T, 2, axis=0)
    sin2s = np.repeat(np.asarray(freqs_sin, np.float32).T, 2, axis=0)
    sin2s[0::2] *= -1.0
    cs2 = np.ascontiguousarray(np.concatenate([cos2, sin2s], axis=0)).astype(bf16)
    Wq = np.asarray(Wq, np.float32)
    Wk = np.asarray(Wk, np.float32)
    Wv = np.asarray(Wv, np.float32)
    w = np.concatenate([Wq, Wq[:, swap], Wk, Wk[:, swap], Wv], axis=1).astype(bf16)
    w = np.ascontiguousarray(w)
    # mask01[b] laid out [j_in_tile(128), j_tile(NT)]
    mask01 = np.asarray(x_latex_mask != 0, np.float32).reshape(N_CORES, NT, 128)

    in_maps = []
    for b in range(N_CORES):
        in_maps.append(
            {
                "xT": np.ascontiguousarray(np.asarray(x_text_emb[b], np.float32).T).astype(bf16),
                "w": w,
                "cs2": cs2,
                "mask01": np.ascontiguousarray(mask01[b].T),
            }
        )

    res = run_bass_kernel_spmd(nc, in_maps, core_ids=list(range(N_CORES)))
    # out arrives [128, NT, HS] with row t = a*128+p at [p, a, :]
    out = np.stack(
        [
            np.asarray(res.results[b]["out"], np.float32)
            .transpose(1, 0, 2)
            .reshape(T, HS)
            for b in range(N_CORES)
        ],
        axis=0,
    )
    return out
